# revision 12
# baseline (speedup 1.0000x reference)
"""MoE (CMRGroupLayer) Trainium2 kernel.

Strategy (expert-parallel dispatch, per sharding hint):
  - Host computes router top-1 (with jax-on-CPU, bit-matching the reference's
    fp32 logits) and dispatches each token to the core owning its expert.
  - Core e holds expert e's W1/W2 plus the shared FFN + CMR gate (replicated),
    and computes EVERYTHING for its gathered tokens: gate, softmax prob of its
    own expert, expert FFN, shared FFN, and the gated mix. No collectives.
  - Device data layout is transposed (xT [D, C]) so the chain
    xT -> hT = relu(W1'.T x + b) -> yT = W2'.T h needs no transposes: the
    natural [D,F] / [F,D] weight layouts ARE the lhsT operands.
  - The expert FFN and shared FFN are stacked along F (K = 2*4096), with the
    per-token scales  a_e = gate*prob  and  a_f = 1-gate  folded into hT's
    columns, so one SBUF accumulator yields the final mixed output directly.
    The W2-side biases enter through a tiny K=2 matmul against [a_e; a_f].
  - Big matmuls run as float32r (full PE rate for free-dim >= 256).
  - F is processed in blocks of 512 so all weights stream from HBM exactly
    once; partial yT accumulates in SBUF via DVE adds.

Outputs per core: outT [D, C] (mixed output, transposed, capacity-padded),
ce_s [E,1] (sum of softmax probs over this core's real tokens), g_s [1,1]
(sum of gates). Host scatters outT rows back by token index and combines the
scalar stats into l_aux / used_budget / total_budget.
"""

import os
import numpy as np

B, S, D, F, E = 8, 1024, 1024, 4096, 8
P = 128
TOK = B * S
KD = D // P        # 8  K-chunks over D
FT = F // P        # 32 f-tiles per FFN
FB = 4             # f-tiles per F-block (512 columns)
NBLK = 2 * FT // FB  # 16 blocks over the stacked 2*F hidden dim
CH = 512           # row-phase chunk width


def _choose_tiles(maxcount):
    """Pick (N, T) with C = N*T >= maxcount, N in [256..512] (fp32r needs
    free dim >= 256), minimizing padded capacity C, preferring larger N."""
    best = None
    for N in (512, 448, 384, 320, 256):
        T = max(1, -(-maxcount // N))
        C = N * T
        key = (C, -N)
        if best is None or key < best[0]:
            best = (key, (N, T))
    return best[1]


def _build_program(C, N, T):
    import concourse.bass as bass
    import concourse.mybir as mybir
    import concourse.tile as tile
    from concourse import bacc

    fp32 = mybir.dt.float32
    fp32r = mybir.dt.float32r
    AF = mybir.ActivationFunctionType
    OP = mybir.AluOpType
    AX = mybir.AxisListType

    nc = bacc.Bacc("TRN2", target_bir_lowering=False)

    # -------- I/O --------
    xT = nc.dram_tensor("xT", [D, C], fp32r, kind="ExternalInput")
    w1e = nc.dram_tensor("w1e", [D, F], fp32r, kind="ExternalInput")
    w2e = nc.dram_tensor("w2e", [F, D], fp32r, kind="ExternalInput")
    wf1 = nc.dram_tensor("wf1", [D, F], fp32r, kind="ExternalInput")
    wf2 = nc.dram_tensor("wf2", [F, D], fp32r, kind="ExternalInput")
    b1e = nc.dram_tensor("b1e", [F], fp32, kind="ExternalInput")
    bf1 = nc.dram_tensor("bf1", [F], fp32, kind="ExternalInput")
    b2e = nc.dram_tensor("b2e", [D], fp32, kind="ExternalInput")
    bf2 = nc.dram_tensor("bf2", [D], fp32, kind="ExternalInput")
    wr = nc.dram_tensor("wr", [D, E], fp32, kind="ExternalInput")
    wg = nc.dram_tensor("wg", [D, 1], fp32, kind="ExternalInput")
    wgb = nc.dram_tensor("wgb", [1, 1], fp32, kind="ExternalInput")
    esel = nc.dram_tensor("esel", [E, 1], fp32, kind="ExternalInput")
    npad = nc.dram_tensor("npad", [1, C], fp32, kind="ExternalInput")
    valid = nc.dram_tensor("valid", [1, C], fp32, kind="ExternalInput")
    outT = nc.dram_tensor("outT", [D, C], fp32, kind="ExternalOutput")
    ce_s = nc.dram_tensor("ce_s", [E, 1], fp32, kind="ExternalOutput")
    g_s = nc.dram_tensor("g_s", [1, 1], fp32, kind="ExternalOutput")

    w2_bufs = 2 if C <= 1200 else 1

    with tile.TileContext(nc) as tc:
        with (
            tc.tile_pool(name="persist", bufs=1) as persist,
            tc.tile_pool(name="w1pool", bufs=2) as w1pool,
            tc.tile_pool(name="w2pool", bufs=w2_bufs) as w2pool,
            tc.tile_pool(name="hpool", bufs=2) as hpool,
            tc.tile_pool(name="rowp", bufs=1) as rowp,
            tc.tile_pool(name="psmain", bufs=2, space="PSUM") as psmain,
            tc.tile_pool(name="psrow", bufs=1, space="PSUM") as psrow,
            tc.tile_pool(name="dramp", bufs=2, space="DRAM") as dramp,
        ):
            # ================= persistent loads =================
            xt = persist.tile([P, KD, C], fp32r)
            for k in range(KD):
                nc.sync.dma_start(out=xt[:, k, :], in_=xT[k * P:(k + 1) * P, :])

            # router + gate weights fused: cols 0..7 = Wr, col 8 = wg
            wrg = persist.tile([P, KD, E + 1], fp32)
            nc.sync.dma_start(
                out=wrg[:, :, 0:E], in_=wr[:].rearrange("(ko p) e -> p ko e", p=P)
            )
            nc.sync.dma_start(
                out=wrg[:, :, E:E + 1], in_=wg[:].rearrange("(ko p) o -> p ko o", p=P)
            )
            # ones / expert-select fused: col 0 = ones, col 1 = esel
            se = persist.tile([E, 2], fp32)
            nc.vector.memset(se[:, 0:1], 1.0)
            nc.sync.dma_start(out=se[:, 1:2], in_=esel[:, :])

            b1_sb = persist.tile([P, 2 * FT], fp32)
            nc.sync.dma_start(out=b1_sb[:, 0:FT], in_=b1e[:].rearrange("(t p) -> p t", p=P))
            nc.sync.dma_start(out=b1_sb[:, FT:2 * FT], in_=bf1[:].rearrange("(t p) -> p t", p=P))
            b2_sb = persist.tile([2, D], fp32)
            nc.sync.dma_start(out=b2_sb[0:1, :], in_=b2e[:][None, :])
            nc.sync.dma_start(out=b2_sb[1:2, :], in_=bf2[:][None, :])

            np_row = persist.tile([1, C], fp32)
            nc.sync.dma_start(out=np_row[:], in_=npad[:, :])
            vl_row = persist.tile([1, C], fp32)
            nc.sync.dma_start(out=vl_row[:], in_=valid[:, :])
            wgb_sb = persist.tile([1, 1], fp32)
            nc.sync.dma_start(out=wgb_sb[:], in_=wgb[:, :])

            gg = persist.tile([2, C], fp32)        # row0 = a_e, row1 = a_f
            ae_row = persist.tile([1, C], fp32)
            af_row = persist.tile([1, C], fp32)
            abc = persist.tile([P, 2, C], fp32)    # a_e/a_f broadcast to 128 parts
            ce_acc = persist.tile([E, 1], fp32)
            nc.vector.memset(ce_acc, 0.0)
            g_acc = persist.tile([1, 1], fp32)
            nc.vector.memset(g_acc, 0.0)
            yacc = persist.tile([P, KD, C], fp32)  # output accumulator (yT)

            # ================= row phase (gates / probs) =================
            for c0 in range(0, C, CH):
                cw = min(CH, C - c0)
                sl = slice(c0, c0 + cw)
                # router logits -> psum [8, cw]
                ps_l = psrow.tile([E, CH], fp32, tag="psl")
                for k in range(KD):
                    nc.tensor.matmul(
                        ps_l[:, :cw], wrg[:, k, 0:E], xt[:, k, sl].bitcast(fp32),
                        start=(k == 0), stop=(k == KD - 1),
                    )
                # gate pre-activation -> psum [1, cw] (partition-0 output)
                ps_g = psrow.tile([1, CH], fp32, tag="psr1")
                for k in range(KD):
                    nc.tensor.matmul(
                        ps_g[:, :cw], wrg[:, k, E:E + 1], xt[:, k, sl].bitcast(fp32),
                        start=(k == 0), stop=(k == KD - 1),
                    )
                pu = rowp.tile([E, CH], fp32, tag="pu")
                nc.scalar.activation(pu[:, :cw], ps_l[0:E, :cw], AF.Exp)
                g_c = rowp.tile([1, CH], fp32, tag="g_c")
                nc.scalar.activation(
                    g_c[:, :cw], ps_g[0:1, :cw], AF.Sigmoid, bias=wgb_sb[:]
                )
                nc.vector.tensor_mul(g_c[:, :cw], g_c[:, :cw], np_row[:, sl])
                # softmax denominator (ones) and own-expert numerator (esel)
                ps_den = psrow.tile([1, CH], fp32, tag="psr1")
                nc.tensor.matmul(ps_den[:, :cw], se[:, 0:1], pu[:, :cw], start=True, stop=True)
                rden = rowp.tile([1, CH], fp32, tag="rden")
                nc.vector.reciprocal(rden[:, :cw], ps_den[0:1, :cw])
                ps_sel = psrow.tile([1, CH], fp32, tag="psr1")
                nc.tensor.matmul(ps_sel[:, :cw], se[:, 1:2], pu[:, :cw], start=True, stop=True)
                # a_e = g * prob_own = g * pu_own * rden
                nc.vector.tensor_mul(ae_row[:, sl], ps_sel[0:1, :cw], rden[:, :cw])
                nc.vector.tensor_mul(ae_row[:, sl], ae_row[:, sl], g_c[:, :cw])
                # a_f = 1 - g
                nc.vector.tensor_scalar(
                    af_row[:, sl], g_c[:, :cw], -1.0, 1.0, OP.mult, OP.add
                )
                # ce partial: sum_t probs[e,t] * valid[t]
                rn = rowp.tile([1, CH], fp32, tag="rn")
                nc.vector.tensor_mul(rn[:, :cw], rden[:, :cw], vl_row[:, sl])
                scr_rn = dramp.tile([1, CH], fp32, tag="scr_rn")
                nc.sync.dma_start(out=scr_rn[:, :cw], in_=rn[:, :cw])
                rn8 = rowp.tile([E, CH], fp32, tag="rn8")
                bcr = bass.AP(
                    tensor=scr_rn.tensor, offset=scr_rn.offset, ap=[[0, E], [1, cw]]
                )
                nc.gpsimd.dma_start(out=rn8[:, :cw], in_=bcr)
                pr8 = rowp.tile([E, CH], fp32, tag="pr8")
                nc.vector.tensor_mul(pr8[:, :cw], pu[:, :cw], rn8[:, :cw])
                ce_c = rowp.tile([E, 1], fp32, tag="ce_c")
                nc.vector.reduce_sum(ce_c, pr8[:, :cw], axis=AX.X)
                nc.vector.tensor_add(ce_acc, ce_acc, ce_c)
                gs_c = rowp.tile([1, 1], fp32, tag="gs_c")
                nc.vector.reduce_sum(gs_c, g_c[:, :cw], axis=AX.X)
                nc.vector.tensor_add(g_acc, g_acc, gs_c)

            nc.sync.dma_start(out=ce_s[:, :], in_=ce_acc[:])
            nc.sync.dma_start(out=g_s[:, :], in_=g_acc[:])

            # assemble gg [2, C] = [a_e; a_f] (DMA writes may target any partition)
            nc.gpsimd.dma_start(out=gg[0:1, :], in_=ae_row[:])
            nc.gpsimd.dma_start(out=gg[1:2, :], in_=af_row[:])
            # broadcast a_e / a_f across all 128 partitions via DRAM bounce
            scr_gg = dramp.tile([2, C], fp32, tag="scr_gg", bufs=1)
            nc.sync.dma_start(out=scr_gg[0:1, :], in_=ae_row[:])
            nc.sync.dma_start(out=scr_gg[1:2, :], in_=af_row[:])
            for h in range(2):
                row = scr_gg[h:h + 1, :]
                bc = bass.AP(tensor=row.tensor, offset=row.offset, ap=[[0, P], [1, C]])
                nc.gpsimd.dma_start(out=abc[:, h, :], in_=bc)

            # ================= main FFN loop =================
            for fb in range(NBLK):
                half = 0 if fb < NBLK // 2 else 1
                src1 = w1e if half == 0 else wf1
                src2 = w2e if half == 0 else wf2
                fcol = (fb % (NBLK // 2)) * (FB * P)
                w1blk = w1pool.tile([P, KD, FB * P], fp32r, tag="w1blk")
                for k in range(KD):
                    nc.sync.dma_start(
                        out=w1blk[:, k, :],
                        in_=src1[k * P:(k + 1) * P, fcol:fcol + FB * P],
                    )
                w2blk = w2pool.tile([P, FB, D], fp32r, tag="w2blk")
                for j in range(FB):
                    nc.sync.dma_start(
                        out=w2blk[:, j, :],
                        in_=src2[fcol + j * P:fcol + (j + 1) * P, :],
                    )
                for tn in range(T):
                    tns = slice(tn * N, (tn + 1) * N)
                    hblk = hpool.tile([P, FB, N], fp32r, tag="hblk")
                    for ft in range(FB):
                        gf = fb * FB + ft
                        ps_h = psmain.tile([P, N], fp32, tag="ph")
                        for k in range(KD):
                            nc.tensor.matmul(
                                ps_h[:],
                                w1blk[:, k, ft * P:(ft + 1) * P],
                                xt[:, k, tns],
                                start=(k == 0), stop=(k == KD - 1),
                            )
                        nc.scalar.activation(
                            hblk[:, ft, :], ps_h[:], AF.Relu, bias=b1_sb[:, gf:gf + 1]
                        )
                        nc.vector.tensor_mul(
                            hblk[:, ft, :], hblk[:, ft, :], abc[:, half, tns]
                        )
                    for dt in range(KD):
                        ps_y = psmain.tile([P, N], fp32, tag="py", bufs=4)
                        for ft in range(FB):
                            nc.tensor.matmul(
                                ps_y[:],
                                w2blk[:, ft, dt * P:(dt + 1) * P],
                                hblk[:, ft, :],
                                start=(ft == 0), stop=(ft == FB - 1),
                            )
                        if fb == 0:
                            nc.vector.tensor_copy(yacc[:, dt, tns], ps_y[:])
                        else:
                            nc.vector.tensor_add(
                                yacc[:, dt, tns], yacc[:, dt, tns], ps_y[:]
                            )

            # ================= epilogue: W2-side biases + store =================
            for dt in range(KD):
                for tn in range(T):
                    tns = slice(tn * N, (tn + 1) * N)
                    ps_b = psmain.tile([P, N], fp32, tag="py", bufs=4)
                    nc.tensor.matmul(
                        ps_b[:],
                        b2_sb[0:2, dt * P:(dt + 1) * P],
                        gg[0:2, tns],
                        start=True, stop=True,
                    )
                    nc.vector.tensor_add(yacc[:, dt, tns], yacc[:, dt, tns], ps_b[:])
                nc.sync.dma_start(out=outT[dt * P:(dt + 1) * P, :], in_=yacc[:, dt, :])

    nc.compile()
    return nc


def kernel(x, lang_idx, input_padding_mask, wg_w, wg_b, Wr, W1, B1, W2, B2,
           Wf1, bf1, Wf2, bf2):
    x = np.ascontiguousarray(np.asarray(x, dtype=np.float32))
    input_padding_mask = np.asarray(input_padding_mask)
    wg_w = np.ascontiguousarray(np.asarray(wg_w, dtype=np.float32))
    wg_b = np.asarray(wg_b, dtype=np.float32)
    Wr = np.ascontiguousarray(np.asarray(Wr, dtype=np.float32))
    W1 = np.asarray(W1, dtype=np.float32)
    B1 = np.asarray(B1, dtype=np.float32)
    W2 = np.asarray(W2, dtype=np.float32)
    B2 = np.asarray(B2, dtype=np.float32)
    Wf1 = np.ascontiguousarray(np.asarray(Wf1, dtype=np.float32))
    bf1 = np.ascontiguousarray(np.asarray(bf1, dtype=np.float32))
    Wf2 = np.ascontiguousarray(np.asarray(Wf2, dtype=np.float32))
    bf2 = np.ascontiguousarray(np.asarray(bf2, dtype=np.float32))

    # ---- routing on host: top-1 of fp32 logits, bit-matching the reference
    # (same jnp einsum/argmax on the default jax platform). This IS the
    # sharding decision for the expert-parallel dispatch.
    import time

    import jax.numpy as jnp

    def _routing():
        logits = jnp.einsum("bsd,de->bse", jnp.asarray(x), jnp.asarray(Wr))
        return np.asarray(jnp.argmax(logits, axis=-1)).reshape(TOK)

    top1 = None
    for attempt in range(4):
        try:
            top1 = _routing()
            break
        except Exception:
            if attempt == 3:
                raise
            time.sleep(10)

    x2d = x.reshape(TOK, D)
    nonpad = (~input_padding_mask).astype(np.float32).reshape(TOK)
    idx = [np.nonzero(top1 == e)[0] for e in range(E)]
    counts = [len(i) for i in idx]
    N, T = _choose_tiles(max(max(counts), 256))
    C = N * T

    nc = _build_program(C, N, T)

    in_maps = []
    for e in range(E):
        cnt = counts[e]
        xg = np.zeros((C, D), np.float32)
        xg[:cnt] = x2d[idx[e]]
        npad_g = np.zeros((1, C), np.float32)
        npad_g[0, :cnt] = nonpad[idx[e]]
        valid_g = np.zeros((1, C), np.float32)
        valid_g[0, :cnt] = 1.0
        onehot = np.zeros((E, 1), np.float32)
        onehot[e, 0] = 1.0
        in_maps.append({
            "xT": np.ascontiguousarray(xg.T),
            "w1e": np.ascontiguousarray(W1[e]),
            "w2e": np.ascontiguousarray(W2[e]),
            "wf1": Wf1,
            "wf2": Wf2,
            "b1e": np.ascontiguousarray(B1[e]),
            "bf1": bf1,
            "b2e": np.ascontiguousarray(B2[e]),
            "bf2": bf2,
            "wr": Wr,
            "wg": wg_w,
            "wgb": wg_b.reshape(1, 1),
            "esel": onehot,
            "npad": npad_g,
            "valid": valid_g,
        })

    from concourse.bass_utils import run_bass_kernel_spmd
    trace = os.environ.get("MOE_KERNEL_TRACE", "0") == "1"
    res = None
    for attempt in range(3):
        try:
            res = run_bass_kernel_spmd(nc, in_maps, core_ids=list(range(E)), trace=trace)
            break
        except Exception:
            if attempt == 2:
                raise
            time.sleep(15)
    if trace and res.exec_time_ns is not None:
        print(f"HW exec time: {res.exec_time_ns} ns")
        print(f"mean exec time: {res.mean_exec_time_ns} ns")
        if res.instructions_and_trace is not None:
            print(f"trace: {res.instructions_and_trace[1]}")

    out2d = np.zeros((TOK, D), np.float32)
    ce_sum = np.zeros(E, np.float32)
    g_sum = np.float32(0.0)
    for e in range(E):
        r = res.results[e]
        if counts[e]:
            out2d[idx[e]] = r["outT"][:, :counts[e]].T
        ce_sum = ce_sum + r["ce_s"][:, 0]
        g_sum = g_sum + r["g_s"][0, 0]

    me = np.asarray(counts, np.float32) / np.float32(TOK)
    ce = ce_sum / np.float32(TOK)
    l_aux = np.float32(E) * np.float32(np.sum(me * ce))
    used_budget = np.float32(g_sum)
    total_budget = np.float32(nonpad.sum())
    x_out = out2d.reshape(B, S, D)
    return x_out, np.float32(l_aux), used_budget, total_budget
